# revision 3
# baseline (speedup 1.0000x reference)
"""Trainium2 Bass kernel for nn_CrossAttention (N=16,Q=4096,C=77,D=512,Dc=768,H=8,S=64).

Sharding: data-parallel over batch N across 8 cores (2 batches/core, no collectives).

Per-chunk structure (CHUNK=512 query rows), all matmul operands bf16:
  q_raw[i,d]    <- gpsimd cast-DMA (f32 dram -> bf16 sbuf)
  queryT[d,i]   <- DMA XBAR transpose (off the PE engine)
  qT[s2,hp,i]   <- Wq_pair.T @ queryT          (PE, N=512)
  scoresT[c,i]  <- kT_h.T @ qT_h               (PE, N=512)
  expT[c,h,i]   <- exp(scoresT * 1/sqrt(S))    (Act, scale folded into exp)
  av_nat[i,65]  <- expT_h_ib.T @ [v_h | 1]     (PE, N=65: av cols 0:64, colsum col 64)
  attn_nat      <- av * recip(colsum)          (DVE: strided recip + stride-0 bcast mult)
  attnT[hs,i]   <- DMA XBAR transpose of attn_nat
  out[i,d]      <- attnT.T @ Wo                (PE, N=512; emitted one chunk behind to
                                                hide the attn transpose latency)

Cost model: matmul = out_free_size cycles (bf16, 1 cyc/row); this removes the
baseline's colsum broadcast matmuls (4096 cyc/chunk), PE transposes (2048), and
shrinks av from 4096 to 2080 cyc/chunk.
"""

import sys

if "/opt/trn_rl_repo" not in sys.path:
    sys.path.insert(0, "/opt/trn_rl_repo")

import numpy as np

import concourse.bass as bass
import concourse.tile as tile
from concourse import bacc, mybir
from concourse.bass_utils import run_bass_kernel_spmd

# Problem shapes (hardcoded per spec)
N, Q, C = 16, 4096, 77
D, DC, H, S = 512, 768, 8, 64
HS = H * S  # 512
N_CORES = 8
NB = N // N_CORES  # batches per core = 2
P = 128
CHUNK = 512
N_CHUNKS = Q // CHUNK  # 8
IT = CHUNK // P  # 4 i-tiles per chunk
N_PAIRS = H // 2  # 4
KT_D = D // P  # 4
KT_DC = DC // P  # 6
CPADT = 80  # ctx rows padded to /16 for DMA transpose
VA = S + 1  # 65: v columns + ones column

F32 = mybir.dt.float32
BF16 = mybir.dt.bfloat16


def build_kernel(use_f32r=True, with_bias=True, pools=None):
    nc = bacc.Bacc("TRN2", target_bir_lowering=False, debug=False,
                   num_devices=N_CORES)

    query = nc.dram_tensor("query", [NB, Q, D], F32, kind="ExternalInput").ap()
    context = nc.dram_tensor("context", [NB, C, DC], F32, kind="ExternalInput").ap()
    Wq = nc.dram_tensor("Wq", [D, HS], F32, kind="ExternalInput").ap()
    Wk = nc.dram_tensor("Wk", [DC, HS], F32, kind="ExternalInput").ap()
    Wv = nc.dram_tensor("Wv", [DC, HS], F32, kind="ExternalInput").ap()
    Wo = nc.dram_tensor("Wo", [HS, D], F32, kind="ExternalInput").ap()
    bo = nc.dram_tensor("bo", [D], F32, kind="ExternalInput").ap()
    out = nc.dram_tensor("out", [NB, Q, D], F32, kind="ExternalOutput").ap()

    with tile.TileContext(nc) as tc:
        _emit(nc, tc, query, context, Wq, Wk, Wv, Wo, bo, out, with_bias,
              pools or {})
    nc.compile()
    return nc


def _emit(nc, tc, query, context, Wq, Wk, Wv, Wo, bo, out, with_bias, pools):
    from contextlib import ExitStack

    pg = lambda k, d: pools.get(k, d)
    scale = float(S) ** -0.5

    ctx = ExitStack()
    with ctx:
        consts = ctx.enter_context(tc.tile_pool(name="consts", bufs=1))
        wpool = ctx.enter_context(tc.tile_pool(name="weights", bufs=1))
        ctxp = ctx.enter_context(tc.tile_pool(name="ctxphase", bufs=1))
        qin = ctx.enter_context(tc.tile_pool(name="qin", bufs=pg("qin", 3)))
        qtp = ctx.enter_context(tc.tile_pool(name="qtp", bufs=pg("qtp", 2)))
        qtc = ctx.enter_context(tc.tile_pool(name="qtc", bufs=pg("qtc", 2)))
        expp = ctx.enter_context(tc.tile_pool(name="expp", bufs=pg("expp", 2)))
        anp = ctx.enter_context(tc.tile_pool(name="attnat", bufs=pg("anp", 6)))
        rcpp = ctx.enter_context(tc.tile_pool(name="rcp", bufs=pg("rcpp", 4)))
        atp = ctx.enter_context(tc.tile_pool(name="attnT", bufs=pg("atp", 2)))
        outp = ctx.enter_context(tc.tile_pool(name="outp", bufs=pg("outp", 2)))

        ps_qp = ctx.enter_context(tc.tile_pool(name="ps_qp", bufs=pg("qp", 2), space="PSUM"))
        ps_sc = ctx.enter_context(tc.tile_pool(name="ps_sc", bufs=pg("sc", 2), space="PSUM"))
        ps_av = ctx.enter_context(tc.tile_pool(name="ps_av", bufs=pg("av", 2), space="PSUM"))
        ps_o = ctx.enter_context(tc.tile_pool(name="ps_o", bufs=pg("o", 2), space="PSUM"))

        # ---- upfront DMAs: ctx b0, wk, wv, q0, wq, ctx b1, wo (gpsimd cast) ----
        ctx_bf = []
        for b in range(NB):
            t = ctxp.tile([CPADT, DC], BF16, tag=f"ctxbf{b}", name=f"ctxbf{b}")
            ctx_bf.append(t)
        nc.gpsimd.memset(ctx_bf[0][C:CPADT, :], 0.0)
        nc.gpsimd.dma_start(ctx_bf[0][:C, :], context[0])

        wk_sb = wpool.tile([P, KT_DC, HS], BF16)
        wv_sb = wpool.tile([P, KT_DC, HS], BF16)
        wq_sb = wpool.tile([P, KT_D, HS], BF16)
        wo_sb = wpool.tile([P, KT_D, D], BF16)
        nc.gpsimd.dma_start(wk_sb[:], Wk.rearrange("(kt p) n -> p kt n", p=P))
        nc.gpsimd.dma_start(wv_sb[:], Wv.rearrange("(kt p) n -> p kt n", p=P))

        # first q chunk prefetch
        q_raw0 = qin.tile([P, IT, CHUNK], BF16, tag="q_raw")
        nc.gpsimd.dma_start(
            q_raw0[:], query[0, 0:CHUNK, :].rearrange("(t p) c -> p t c", p=P))

        nc.gpsimd.dma_start(wq_sb[:], Wq.rearrange("(kt p) n -> p kt n", p=P))
        nc.gpsimd.memset(ctx_bf[1][C:CPADT, :], 0.0)
        nc.gpsimd.dma_start(ctx_bf[1][:C, :], context[1])
        nc.gpsimd.dma_start(wo_sb[:], Wo.rearrange("(kt p) n -> p kt n", p=P))

        if with_bias:
            onesrow = consts.tile([1, P], BF16)
            nc.gpsimd.memset(onesrow[:], 1.0)
            bo_sb = consts.tile([1, D], BF16)
            nc.gpsimd.dma_start(bo_sb[:], bo[None, :])

        # ---- ctx phase per batch: ctxT (DMA transpose), kT, v_aug ----
        kT = []     # per batch: [128(s2), N_PAIRS, C] bf16
        v_aug = []  # per batch: [C, H, VA] bf16 (col 64 = ones)
        for b in range(NB):
            ctxT = ctxp.tile([P, KT_DC, CPADT], BF16, tag=f"ctxT{b}", name=f"ctxT{b}")
            nc.sync.dma_start(ctxT[:], ctx_bf[b][:], transpose=True)

            kT_b = ctxp.tile([P, N_PAIRS, C], BF16, tag=f"kT{b}", name=f"kT{b}")
            va_b = ctxp.tile([C, H, VA], BF16, tag=f"vaug{b}", name=f"vaug{b}")
            nc.gpsimd.memset(va_b[:, :, S:VA], 1.0)
            for hp in range(N_PAIRS):
                pk = ps_sc.tile([P, CHUNK], F32, tag="sc")
                for kt in range(KT_DC):
                    nc.tensor.matmul(
                        pk[:, :C],
                        wk_sb[:, kt, hp * P:(hp + 1) * P],
                        ctxT[:, kt, :C],
                        start=(kt == 0), stop=(kt == KT_DC - 1),
                    )
                nc.vector.tensor_copy(kT_b[:, hp, :], pk[:, :C])
                pv = ps_av.tile([P, CHUNK], F32, tag="av")
                for kt in range(KT_DC):
                    nc.tensor.matmul(
                        pv[:C, :P],
                        ctxT[:, kt, :C],
                        wv_sb[:, kt, hp * P:(hp + 1) * P],
                        start=(kt == 0), stop=(kt == KT_DC - 1),
                    )
                # pv[:, 0:64] -> v_aug[:, 2hp, 0:64]; pv[:, 64:128] -> v_aug[:, 2hp+1, 0:64]
                nc.vector.tensor_copy(
                    va_b[:, 2 * hp:2 * hp + 2, 0:S],
                    pv[:C, :P].rearrange("c (h s) -> c h s", h=2),
                )
            kT.append(kT_b)
            v_aug.append(va_b)

        # ---- main loop: o-proj runs one chunk behind ----
        total = NB * N_CHUNKS
        pending = None  # (b, ch, attnT_tile)

        def emit_oproj(b, ch, attnT_c):
            outc = outp.tile([P, IT, D], F32, tag="outc")
            for it in range(IT):
                po = ps_o.tile([P, D], F32, tag="o")
                for kt in range(KT_D):
                    nc.tensor.matmul(
                        po[:],
                        attnT_c[:, kt, it * P:(it + 1) * P],
                        wo_sb[:, kt, :],
                        start=(kt == 0),
                        stop=(not with_bias and kt == KT_D - 1),
                    )
                if with_bias:
                    nc.tensor.matmul(po[:], onesrow[:], bo_sb[:],
                                     start=False, stop=True)
                nc.vector.tensor_copy(outc[:, it, :], po[:])
            nc.sync.dma_start(
                out[b, ch * CHUNK:(ch + 1) * CHUNK, :]
                .rearrange("(t p) c -> p t c", p=P),
                outc[:],
            )

        q_raw = q_raw0
        for step in range(total):
            b, ch = divmod(step, N_CHUNKS)

            # prefetch next chunk's q_raw
            if step + 1 < total:
                nb_, nch = divmod(step + 1, N_CHUNKS)
                q_next = qin.tile([P, IT, CHUNK], BF16, tag="q_raw")
                nc.gpsimd.dma_start(
                    q_next[:],
                    query[nb_, nch * CHUNK:(nch + 1) * CHUNK, :]
                    .rearrange("(t p) c -> p t c", p=P))
            else:
                q_next = None

            # queryT via DMA XBAR transpose: [p, kt, i] = q_raw[i, kt*128+p]
            queryT_c = qtp.tile([P, KT_D, CHUNK], BF16, tag="queryT")
            for it in range(IT):
                nc.sync.dma_start(
                    queryT_c[:, :, it * P:(it + 1) * P],
                    q_raw[:, it, :],
                    transpose=True,
                )

            # q-proj + scores, interleaved so scores hp trails qproj hp+1
            qT_c = qtc.tile([P, N_PAIRS, CHUNK], BF16, tag="qT")
            ps_list = [None] * (2 * N_PAIRS)

            def emit_qproj(hp):
                pq = ps_qp.tile([P, CHUNK], F32, tag="qp")
                for kt in range(KT_D):
                    nc.tensor.matmul(
                        pq[:],
                        wq_sb[:, kt, hp * P:(hp + 1) * P],
                        queryT_c[:, kt, :],
                        start=(kt == 0), stop=(kt == KT_D - 1),
                    )
                nc.scalar.copy(qT_c[:, hp, :], pq[:])

            def emit_scores(hp):
                ps0 = ps_sc.tile([P, CHUNK], F32, tag="sc")
                ps1 = ps_sc.tile([P, CHUNK], F32, tag="sc")
                nc.tensor.matmul(ps0[:C, :], kT[b][0:S, hp, :],
                                 qT_c[0:S, hp, :], start=True, stop=True)
                nc.tensor.matmul(ps1[:C, :], kT[b][S:P, hp, :],
                                 qT_c[S:P, hp, :], start=True, stop=True)
                ps_list[2 * hp] = ps0
                ps_list[2 * hp + 1] = ps1

            expT_c = expp.tile([C, H, CHUNK], BF16, tag="expT")

            def emit_exp(hp):
                for hh in range(2):
                    h = 2 * hp + hh
                    nc.scalar.activation(
                        expT_c[:, h, :], ps_list[2 * hp + hh][:C, :],
                        mybir.ActivationFunctionType.Exp, scale=scale,
                    )

            emit_qproj(0)
            emit_qproj(1)
            emit_scores(0)
            emit_exp(0)
            emit_qproj(2)
            emit_scores(1)
            emit_exp(1)
            emit_qproj(3)
            emit_scores(2)
            emit_exp(2)
            emit_scores(3)
            emit_exp(3)

            # o-proj of the previous chunk fills PE time while exps/avs settle
            if pending is not None:
                emit_oproj(*pending)

            # av (flipped, N=65) + normalize + attnT DMA transpose
            attnT_c = atp.tile([P, KT_D, CHUNK], BF16, tag="attnT")
            for ib in range(IT):
                pavA = ps_av.tile([P, 4 * VA], F32, tag="av")
                pavB = ps_av.tile([P, 4 * VA], F32, tag="av")
                for h in range(H):
                    pav = pavA if h < 4 else pavB
                    g = h % 4
                    nc.tensor.matmul(
                        pav[:, g * VA:(g + 1) * VA],
                        expT_c[:, h, ib * P:(ib + 1) * P],
                        v_aug[b][:, h, :],
                        start=True, stop=True,
                    )
                attn_nat = anp.tile([P, HS], BF16, tag="attn_nat")
                for half, pav in ((0, pavA), (1, pavB)):
                    rcp = rcpp.tile([P, 4], F32, tag="rcp")
                    grp = pav[:].rearrange("p (g c) -> p g c", g=4)
                    nc.vector.reciprocal(rcp[:], grp[:, :, S])
                    nc.vector.tensor_tensor(
                        attn_nat[:, half * 256:(half + 1) * 256]
                        .rearrange("p (g c) -> p g c", g=4),
                        grp[:, :, 0:S],
                        rcp[:].unsqueeze(2).to_broadcast((P, 4, S)),
                        mybir.AluOpType.mult,
                    )
                nc.sync.dma_start(
                    attnT_c[:, :, ib * P:(ib + 1) * P],
                    attn_nat[:],
                    transpose=True,
                )

            pending = (b, ch, attnT_c)
            q_raw = q_next

        emit_oproj(*pending)


_CACHE = {}


def _get_nc(use_f32r=True, with_bias=True):
    key = (use_f32r, with_bias)
    if key not in _CACHE:
        _CACHE[key] = build_kernel(use_f32r, with_bias)
    return _CACHE[key]


def kernel(query, context, Wq, Wk, Wv, Wo, bo, _use_f32r=True):
    query = np.ascontiguousarray(np.asarray(query, dtype=np.float32))
    context = np.ascontiguousarray(np.asarray(context, dtype=np.float32))
    Wq = np.asarray(Wq, dtype=np.float32).reshape(D, HS)
    Wk = np.asarray(Wk, dtype=np.float32).reshape(DC, HS)
    Wv = np.asarray(Wv, dtype=np.float32).reshape(DC, HS)
    Wo = np.asarray(Wo, dtype=np.float32).reshape(HS, D)
    bo = np.asarray(bo, dtype=np.float32).reshape(D)

    nc = _get_nc(use_f32r=_use_f32r, with_bias=bool(np.any(bo)))
    in_maps = []
    for c in range(N_CORES):
        sl = slice(c * NB, (c + 1) * NB)
        in_maps.append({
            "query": np.ascontiguousarray(query[sl]),
            "context": np.ascontiguousarray(context[sl]),
            "Wq": Wq, "Wk": Wk, "Wv": Wv, "Wo": Wo, "bo": bo,
        })
    res = run_bass_kernel_spmd(nc, in_maps, core_ids=list(range(N_CORES)))
    return np.concatenate([res.results[c]["out"] for c in range(N_CORES)], axis=0)
